# revision 1
# baseline (speedup 1.0000x reference)
"""Trainium2 Bass kernel for nn_CrossAttention (B=8, L=1024, QD=1024, KVD=768, H=16).

Sharding: data-parallel over batch across the 8 NeuronCores (1 batch row each).
Per-core pipeline (all bf16 matmuls, fp32 accumulation / residual / layernorm):
  A) cast fp32->bf16 HBM->SBUF on SWDGE (read-bound), store bf16 SBUF->DRAM on
     HWDGE, then big DMA-xbar transposes DRAM->SBUF into "transposed world"
     layouts (contraction dim on partitions).
  B) projections: qhT/khT (transposed, per-partition bias via tensor_scalar),
     vh natural with ones-augmented columns (bias via rank-1 matmul). B1/B2 run
     as a prefix; B3 (vh) is interleaved into the first attention pairs so the
     ACT exp stream starts as early as possible.
  C) attention per head pair: scoresT = khT.T @ qhT (two heads row-packed on
     the PE), exp with mask+scale folded into the ACT pass, attnV with
     [ones|vh] stationary giving psum rows 0:64 = replicated denominator and
     rows 64:128 = o; fast approx reciprocal + multiply on DVE. attnV lags one
     pair behind scores+exp so the in-order PE queue never stalls on exp.
  D) out-projection from oT stationary + rank-1 bias, fp32 residual + layernorm.
"""

import numpy as np

import concourse.bass as bass
import concourse.mybir as mybir
import concourse.tile as tile
from concourse import bacc
from concourse.bass_utils import run_bass_kernel_spmd

F32 = mybir.dt.float32
BF16 = mybir.dt.bfloat16
U8 = mybir.dt.uint8

B = 8
L = 1024
C = 1024      # QD
KV = 768      # KVD
H = 16
DH = 64
P = 128
LT = L // P          # 8 l-tiles
CT = C // P          # 8 contraction tiles (model dim)
KT = KV // P         # 6 contraction tiles (kv dim)
DT = C // P          # 8 d-tiles
NH = C // 512        # 2 free-dim halves (N=512 per PSUM bank)
SCALE = DH ** -0.5
EPS = 1e-5
MASK_NEG = -30000.0

Exp = mybir.ActivationFunctionType.Exp
Sqrt = mybir.ActivationFunctionType.Sqrt
Identity = mybir.ActivationFunctionType.Identity
MULT = mybir.AluOpType.mult
ADD = mybir.AluOpType.add

TRACE = False
LAST_RESULT = None
_CACHE = {}


def _bcast_ap(handle, parts):
    apx = handle[:]
    return bass.AP(tensor=apx.tensor, offset=apx.offset,
                   ap=[[0, parts]] + [list(x) for x in apx.ap])


def build(apply_gb=False):
    nc = bacc.Bacc("TRN2", target_bir_lowering=False)

    q_in = nc.dram_tensor("q", [L, C], F32, kind="ExternalInput")
    k_in = nc.dram_tensor("k", [L, KV], F32, kind="ExternalInput")
    v_in = nc.dram_tensor("v", [L, KV], F32, kind="ExternalInput")
    m_in = nc.dram_tensor("key_padding_mask", [L], U8, kind="ExternalInput")
    wq_in = nc.dram_tensor("Wq", [C, C], F32, kind="ExternalInput")
    bq_in = nc.dram_tensor("bq", [C], F32, kind="ExternalInput")
    wk_in = nc.dram_tensor("Wk", [C, KV], F32, kind="ExternalInput")
    bk_in = nc.dram_tensor("bk", [C], F32, kind="ExternalInput")
    wv_in = nc.dram_tensor("Wv", [C, KV], F32, kind="ExternalInput")
    bv_in = nc.dram_tensor("bv", [C], F32, kind="ExternalInput")
    wo_in = nc.dram_tensor("Wo", [C, C], F32, kind="ExternalInput")
    bo_in = nc.dram_tensor("bo", [C], F32, kind="ExternalInput")
    gamma_in = nc.dram_tensor("gamma", [C], F32, kind="ExternalInput")
    beta_in = nc.dram_tensor("beta", [C], F32, kind="ExternalInput")
    y_out = nc.dram_tensor("y", [L, C], F32, kind="ExternalOutput")

    with tile.TileContext(nc) as tc:
        with (
            tc.tile_pool(name="dram", bufs=1, space="DRAM") as dram,
            tc.tile_pool(name="cst", bufs=1) as cst,
            tc.tile_pool(name="persist", bufs=1) as persist,
        ):
            dram_bf = {}
            stg_pool = [None]

            def _flat(ap2d):
                return ap2d.rearrange("r c -> (r c)").rearrange("(p n) -> p n", p=P)

            def cast_bounce(nm, hnd, rows, cols):
                # Cast fp32->bf16 HBM->SBUF on SWDGE (HBM reads only; DRAM->DRAM
                # halves effective BW), store bf16 back contiguously on HWDGE.
                # One DMA per step: the 8 DMA sem lanes are the scarce resource.
                t = dram.tile([rows, cols], BF16, name=f"{nm}_bf", tag=f"{nm}_bf")
                dram_bf[nm] = t
                half = rows // 2
                for rh in range(2):
                    rs = slice(rh * half, (rh + 1) * half)
                    st = stg_pool[0].tile([P, half * cols // P], BF16,
                                          name=f"st_{nm}{rh}", tag="stg")
                    nc.gpsimd.dma_start(st, _flat(hnd[rs, :]))
                    nc.sync.dma_start(_flat(t[rs, :]), st)

            def transp(dst, src):
                nc.sync.dma_start(dst, src, transpose=True)

            # ---------------- projection outputs (persist through attention)
            qhT = persist.tile([P, DT, L], BF16)          # d on partitions
            khT = persist.tile([P, DT, L], BF16)
            vh_aug = persist.tile([P, LT, H * P], BF16)   # per m-tile: 16x[64 ones | 64 vh]

            with (
                tc.tile_pool(name="stageK", bufs=1) as stageK,
                tc.tile_pool(name="stg", bufs=3) as stg,
                tc.tile_pool(name="psum_b", bufs=2, space="PSUM") as psum_b,
            ):
                stg_pool[0] = stg
                with tc.tile_pool(name="stageQ", bufs=1) as stageQ:
                    # tiny consts first (they gate B evictions / first exps)
                    bq_sb = cst.tile([P, DT], F32)
                    nc.gpsimd.dma_start(bq_sb, bq_in[:].rearrange("(t p) -> p t", p=P))
                    bk_sb = cst.tile([P, DT], F32)
                    nc.gpsimd.dma_start(bk_sb, bk_in[:].rearrange("(t p) -> p t", p=P))
                    mask_u8 = cst.tile([P, LT], U8)
                    nc.gpsimd.dma_start(mask_u8, m_in[:].rearrange("(t p) -> p t", p=P))
                    mask_bias = cst.tile([P, LT], F32)
                    nc.vector.tensor_copy(mask_bias, mask_u8)
                    nc.vector.tensor_scalar(mask_bias, mask_bias, -MASK_NEG, MASK_NEG,
                                            MULT, ADD)
                    ones_row = cst.tile([1, P], BF16)
                    nc.vector.memset(ones_row, 1.0)
                    eps_sb = cst.tile([P, 1], F32)
                    nc.vector.memset(eps_sb, EPS)
                    # casts for wq/q first — they gate all compute
                    cast_bounce("wq", wq_in, C, C)
                    cast_bounce("q", q_in, L, C)
                    WqT = stageQ.tile([P, CT, C], BF16)
                    qT = stageQ.tile([P, CT, L], BF16)
                    transp(WqT, dram_bf["wq"][:])
                    transp(qT, dram_bf["q"][:])

                    cast_bounce("wk", wk_in, C, KV)
                    cast_bounce("k", k_in, L, KV)
                    WkT = stageK.tile([P, KT, C], BF16)
                    kT = stageK.tile([P, KT, L], BF16)
                    transp(WkT, dram_bf["wk"][:])
                    transp(kT, dram_bf["k"][:])

                    # remaining casts (v path + wo), then constants
                    cast_bounce("wv", wv_in, C, KV)
                    cast_bounce("v", v_in, L, KV)
                    cast_bounce("wo", wo_in, C, C)

                    bv_bf = cst.tile([1, C], BF16)
                    nc.gpsimd.dma_start(bv_bf, bv_in[:].rearrange("(a c) -> a c", a=1))
                    bo_bf = cst.tile([1, C], BF16)
                    nc.gpsimd.dma_start(bo_bf, bo_in[:].rearrange("(a c) -> a c", a=1))
                    if apply_gb:
                        gamma_b = cst.tile([P, C], F32)
                        nc.gpsimd.dma_start(gamma_b, _bcast_ap(gamma_in, P))
                        beta_b = cst.tile([P, C], F32)
                        nc.gpsimd.dma_start(beta_b, _bcast_ap(beta_in, P))
                    else:
                        gamma_b = beta_b = None

                    # ---- B1: qhT[d, l]
                    for dt in range(DT):
                        for lh in range(NH):
                            ps = psum_b.tile([P, 512], F32, tag="ps")
                            for ct in range(CT):
                                nc.tensor.matmul(ps, WqT[:, ct, dt * P:(dt + 1) * P],
                                                 qT[:, ct, lh * 512:(lh + 1) * 512],
                                                 start=(ct == 0), stop=(ct == CT - 1))
                            nc.vector.tensor_scalar_add(
                                qhT[:, dt, lh * 512:(lh + 1) * 512], ps,
                                bq_sb[:, dt:dt + 1])

                # ---- B2: khT[d, l]
                for dt in range(DT):
                    for lh in range(NH):
                        ps = psum_b.tile([P, 512], F32, tag="ps")
                        for ct in range(KT):
                            nc.tensor.matmul(ps, WkT[:, ct, dt * P:(dt + 1) * P],
                                             kT[:, ct, lh * 512:(lh + 1) * 512],
                                             start=(ct == 0), stop=(ct == KT - 1))
                        nc.vector.tensor_scalar_add(
                            khT[:, dt, lh * 512:(lh + 1) * 512], ps,
                            bk_sb[:, dt:dt + 1])

            with tc.tile_pool(name="vstage", bufs=1) as vstage:
                WvT = vstage.tile([P, KT, C], BF16)
                vT = vstage.tile([P, KT, L], BF16)
                transp(WvT, dram_bf["wv"][:])
                transp(vT, dram_bf["v"][:])

                with tc.tile_pool(name="late", bufs=1) as late:
                    WoT = late.tile([P, DT, C], BF16)
                    transp(WoT, dram_bf["wo"][:])
                    oT = late.tile([P, DT, L], BF16)

                    # ---------------- attention, with B3 (vh projection)
                    # interleaved into the first two pair slots
                    with (
                        tc.tile_pool(name="ptp", bufs=26) as ptp,
                        tc.tile_pool(name="recp", bufs=4) as recp,
                        tc.tile_pool(name="psum_sc", bufs=2, space="PSUM") as psum_sc,
                        tc.tile_pool(name="psum_av", bufs=3, space="PSUM") as psum_av,
                        tc.tile_pool(name="psum_b3", bufs=1, space="PSUM") as psum_b3,
                    ):
                        pts = {}

                        def scores_exp(pair):
                            for mt in range(LT):
                                sc = []
                                for hh in range(2):
                                    s = psum_sc.tile([P, L], F32,
                                                     name=f"sc{pair}_{mt}_{hh}", tag="sc")
                                    sc.append(s)
                                    p0 = hh * DH
                                    for lh in range(NH):
                                        nc.tensor.matmul(
                                            s[:, lh * 512:(lh + 1) * 512],
                                            khT[p0:p0 + DH, pair, mt * P:(mt + 1) * P],
                                            qhT[p0:p0 + DH, pair, lh * 512:(lh + 1) * 512],
                                            start=True, stop=True)
                                for hh in range(2):
                                    pt = ptp.tile([P, L], BF16,
                                                  name=f"pt{pair}_{mt}_{hh}", tag="pt")
                                    pts[(pair, mt, hh)] = pt
                                    nc.scalar.activation(pt, sc[hh], Exp,
                                                         bias=mask_bias[:, mt:mt + 1],
                                                         scale=SCALE)

                        def b3_chunk(mts):
                            for mt in mts:
                                for dh2 in range(NH):
                                    ps = psum_b3.tile([P, 512], F32, tag="ps3")
                                    for ct in range(KT):
                                        nc.tensor.matmul(
                                            ps, vT[:, ct, mt * P:(mt + 1) * P],
                                            WvT[:, ct, dh2 * 512:(dh2 + 1) * 512],
                                            start=(ct == 0), stop=False)
                                    nc.tensor.matmul(
                                        ps, ones_row[0:1, :],
                                        bv_bf[0:1, dh2 * 512:(dh2 + 1) * 512],
                                        start=False, stop=True)
                                    dst = vh_aug[:, mt, :].rearrange(
                                        "p (h x) -> p h x", x=P)
                                    dst = dst[:, dh2 * 8:(dh2 + 1) * 8, DH:P]
                                    nc.vector.tensor_copy(
                                        dst, ps[:].rearrange("p (h d) -> p h d", d=DH))

                        def attnv(pair):
                            for hh in range(2):
                                h = 2 * pair + hh
                                avs = [psum_av.tile([P, 512], F32,
                                                    name=f"av{pair}_{hh}_{lh}",
                                                    tag="av")
                                       for lh in range(NH)]
                                # mt-outer: both l-halves reuse each vh stationary load
                                for mt in range(LT):
                                    for lh in range(NH):
                                        nc.tensor.matmul(
                                            avs[lh],
                                            vh_aug[:, mt, h * P:(h + 1) * P],
                                            pts[(pair, mt, hh)][:, lh * 512:(lh + 1) * 512],
                                            start=(mt == 0), stop=(mt == LT - 1))
                                for lh in range(NH):
                                    av = avs[lh]
                                    rec = recp.tile([P, 512], F32,
                                                    name=f"rec{pair}_{hh}_{lh}",
                                                    tag="rec")
                                    nc.vector.reciprocal_approx_fast(rec[0:DH, :],
                                                                     av[0:DH, :])
                                    nc.vector.tensor_mul(
                                        oT[hh * DH:(hh + 1) * DH, pair,
                                           lh * 512:(lh + 1) * 512],
                                        av[DH:P, :], rec[0:DH, :])
                                for mt in range(LT):
                                    del pts[(pair, mt, hh)]

                        nc.vector.memset(vh_aug[:], 1.0)
                        scores_exp(0)
                        b3_chunk(range(0, 4))
                        scores_exp(1)
                        b3_chunk(range(4, 8))
                        attnv(0)
                        for pair in range(2, H // 2):
                            scores_exp(pair)
                            attnv(pair - 1)
                        attnv(H // 2 - 1)

                    # ---------------- out-projection + residual + layernorm
                    with (
                        tc.tile_pool(name="dwork", bufs=3) as dwork,
                        tc.tile_pool(name="dsmall", bufs=8) as dsmall,
                        tc.tile_pool(name="psum_y", bufs=3, space="PSUM") as psum_y,
                    ):
                        qrs = []
                        for lt in range(LT):
                            qr = dwork.tile([P, C], F32, name=f"qr{lt}", tag="qr",
                                            bufs=8)
                            nc.sync.dma_start(qr, q_in[lt * P:(lt + 1) * P, :])
                            qrs.append(qr)
                        for lt in range(LT):
                            yp = psum_y.tile([P, C], F32, tag="yp")
                            for ch in range(NH):
                                for dt in range(DT):
                                    nc.tensor.matmul(
                                        yp[:, ch * 512:(ch + 1) * 512],
                                        oT[:, dt, lt * P:(lt + 1) * P],
                                        WoT[:, dt, ch * 512:(ch + 1) * 512],
                                        start=(dt == 0), stop=False)
                                nc.tensor.matmul(
                                    yp[:, ch * 512:(ch + 1) * 512],
                                    ones_row[0:1, :],
                                    bo_bf[0:1, ch * 512:(ch + 1) * 512],
                                    start=False, stop=True)
                            ysb = dwork.tile([P, C], F32, tag="ysb")
                            nc.vector.tensor_add(ysb, yp, qrs[lt])
                            st = dsmall.tile([P, 2, 6], F32, tag="st")
                            nc.vector.bn_stats(st[:, 0, :], ysb[:, 0:512])
                            nc.vector.bn_stats(st[:, 1, :], ysb[:, 512:1024])
                            mv = dsmall.tile([P, 2], F32, tag="mv")
                            nc.vector.bn_aggr(mv, st)
                            rstd = dsmall.tile([P, 1], F32, tag="rstd")
                            nc.scalar.activation(rstd, mv[:, 1:2], Sqrt,
                                                 bias=eps_sb[:, 0:1])
                            nc.vector.reciprocal(rstd, rstd)
                            nmr = dsmall.tile([P, 1], F32, tag="nmr")
                            nc.vector.tensor_mul(nmr, mv[:, 0:1], rstd)
                            nc.vector.tensor_scalar_mul(nmr, nmr, -1.0)
                            yn = dwork.tile([P, C], F32, tag="yn")
                            nc.scalar.activation(yn, ysb, Identity, bias=nmr[:, 0:1],
                                                 scale=rstd[:, 0:1])
                            if apply_gb:
                                nc.vector.tensor_mul(yn, yn, gamma_b)
                                nc.gpsimd.tensor_add(yn, yn, beta_b)
                            nc.sync.dma_start(y_out[lt * P:(lt + 1) * P, :], yn)

    nc.compile()
    return nc


def _get_nc(apply_gb):
    key = ("nc", apply_gb)
    if key not in _CACHE:
        _CACHE[key] = build(apply_gb)
    return _CACHE[key]


def kernel(**inputs) -> np.ndarray:
    global LAST_RESULT
    gamma = np.asarray(inputs["gamma"], dtype=np.float32)
    beta = np.asarray(inputs["beta"], dtype=np.float32)
    apply_gb = not (np.all(gamma == 1.0) and np.all(beta == 0.0))
    nc = _get_nc(apply_gb)
    q = np.ascontiguousarray(np.asarray(inputs["q"], dtype=np.float32))
    k = np.ascontiguousarray(np.asarray(inputs["k"], dtype=np.float32))
    v = np.ascontiguousarray(np.asarray(inputs["v"], dtype=np.float32))
    mask = np.ascontiguousarray(np.asarray(inputs["key_padding_mask"]).astype(np.uint8))
    shared = {
        name: np.ascontiguousarray(np.asarray(inputs[name], dtype=np.float32))
        for name in ("Wq", "bq", "Wk", "bk", "Wv", "bv", "Wo", "bo", "gamma", "beta")
    }
    in_maps = []
    for b in range(B):
        m = {"q": q[b], "k": k[b], "v": v[b], "key_padding_mask": mask[b]}
        m.update(shared)
        in_maps.append(m)
    LAST_RESULT = run_bass_kernel_spmd(nc, in_maps, core_ids=list(range(B)), trace=TRACE)
    return np.stack([r["y"] for r in LAST_RESULT.results], axis=0)

